# revision 47
# baseline (speedup 1.0000x reference)
"""Multi-head attention (B=2, S=2048, D=1024, H=16) on 8 Trainium2 cores.

Sharding: (batch, head-group-of-4) -> 8 cores, Megatron-style. Core c
handles batch c//4 and heads 4*(c%4)..4*(c%4)+3 (d_local = 256 columns of
Wq/Wk/Wv, 256 rows of Wo). Each core computes a partial [2048, 1024]
output; the host sums the 4 partials per batch (row-parallel Wo).

Key-side truncation: only ceil(max(valid_lens)/128) key tiles are ever
computed; invalid keys get a -1e6 bias on the ScalarE exp (scores are
computed transposed [key, query], denominator rides the ctx matmul as
64 "ones" columns of V'').

This build is organized as one deep pipeline so the ACT-engine exp
stream (the phase-C bottleneck) and the DMA streams hide under the
TensorE matmul stream (the global floor):
  warmup dummy matmuls (p-state ramp) -> K-proj (rides xk DMA) ->
  V-proj (rides per-key-tile xv DMA) -> Q-proj half 0 (rides xq) ->
  attention half 0 (with Q-proj half 1 matmuls injected into the
  per-key-tile slack) -> attention half 1 (with out-proj half 0
  injected) -> out-proj half 1, evacuations spread across ACT/DVE/Pool.

Precision: fp16 streams, fp16 single-pass matmuls, fp32 PSUM; partial
outputs return fp16 and are summed in fp32 on host.
"""
import sys
if "/opt/trn_rl_repo" not in sys.path:
    sys.path.insert(0, "/opt/trn_rl_repo")
import os
import time
import numpy as np

B, SQ, SK, D, H, HD = 2, 2048, 2048, 1024, 16, 64
NEG = -1.0e6
N_CORES = 8
DL = 256          # d_local: 4 heads * 64
KD = D // 128     # contraction tiles over D
N_WARM = int(os.environ.get("BASS_MHA_WARM", "7"))

_NC_CACHE = {}
last_results = None
last_exec_wall_s = None


def _chunks(total, cw):
    out = []
    c0 = 0
    while c0 < total:
        w = min(cw, total - c0)
        out.append((c0, w))
        c0 += w
    return out


def _build(KT):
    import concourse.bass as bass  # noqa: F401
    import concourse.tile as tile
    from concourse import bacc, mybir

    f32 = mybir.dt.float32
    f16 = mybir.dt.float16
    Exp = mybir.ActivationFunctionType.Exp
    LK = KT * 128

    nc = bacc.Bacc("TRN2", target_bir_lowering=False, debug=False,
                   num_devices=N_CORES)
    xqT = nc.dram_tensor("xqT", [D, SQ], f16, kind="ExternalInput")
    xkT = nc.dram_tensor("xkT", [D, LK], f16, kind="ExternalInput")
    xvT = nc.dram_tensor("xvT", [D, LK], f16, kind="ExternalInput")
    wq = nc.dram_tensor("wq", [D, DL], f16, kind="ExternalInput")
    wk = nc.dram_tensor("wk", [D, DL], f16, kind="ExternalInput")
    wv = nc.dram_tensor("wv", [D, DL], f16, kind="ExternalInput")
    wo = nc.dram_tensor("wo", [DL, D], f16, kind="ExternalInput")
    mask = nc.dram_tensor("mask", [128, KT], f32, kind="ExternalInput")
    out = nc.dram_tensor("out", [SQ, D], f16, kind="ExternalOutput")

    with tile.TileContext(nc) as tc:
        with tc.tile_pool(name="sg", bufs=1) as sg:
            wk_sb = sg.tile([128, KD, DL], f16)
            wq_sb = sg.tile([128, KD, DL], f16)
            wv_sb = sg.tile([128, KD, DL], f16)
            wo_sb = sg.tile([128, DL // 128, D], f16)
            mask_sb = sg.tile([128, KT], f32)
            kt_sb = sg.tile([128, 2, LK], f16)
            qt_sb = sg.tile([128, 2, SQ], f16)
            # V'' per head: [key, (v-tile | ones)] pairs per key tile
            v3_sb = sg.tile([128, 4, KT, 2, HD], f16)
            ctxT_sb = sg.tile([128, 2, SQ], f16)
            zero_sb = sg.tile([128, 512], f16)
            xk_sb = sg.tile([128, KD, LK], f16)
            xv_sb = sg.tile([128, KD, KT, 128], f16)
            xq_sb = sg.tile([128, KD, SQ], f16)
            warm_sb = sg.tile([1, 1], f32)

            # ---- DMA queue: arrival order is the pipeline order ----
            def dma_wk(c):
                nc.sync.dma_start(
                    out=wk_sb[:, c * 4:(c + 1) * 4, :],
                    in_=wk[c * 512:(c + 1) * 512, :].rearrange(
                        "(k p) j -> p k j", p=128))

            def dma_xk(c):
                nc.sync.dma_start(
                    out=xk_sb[:, 2 * c:2 * c + 2, :],
                    in_=xkT[c * 256:(c + 1) * 256, :].rearrange(
                        "(k p) j -> p k j", p=128))

            def dma_xq(half, k0, nk):
                nc.sync.dma_start(
                    out=xq_sb[:, k0:k0 + nk,
                              half * 1024:(half + 1) * 1024],
                    in_=xqT[k0 * 128:(k0 + nk) * 128,
                            half * 1024:(half + 1) * 1024].rearrange(
                                "(k p) j -> p k j", p=128))

            nc.sync.dma_start(out=mask_sb, in_=mask[:, :])
            dma_wk(0)
            dma_xk(0)
            dma_xk(1)
            dma_wk(1)
            dma_xk(2)
            nc.sync.dma_start(out=wq_sb,
                              in_=wq[:, :].rearrange("(k p) j -> p k j", p=128))
            dma_xk(3)
            # first two k-tiles arrive singly so phase B can start sooner
            dma_xq(0, 0, 1)
            dma_xq(0, 1, 1)
            for c in range(1, 4):
                dma_xq(0, 2 * c, 2)
            nc.sync.dma_start(out=wv_sb,
                              in_=wv[:, :].rearrange("(k p) j -> p k j", p=128))
            for t in range(KT):
                nc.sync.dma_start(
                    out=xv_sb[:, :, t, :],
                    in_=xvT[:, t * 128:(t + 1) * 128].rearrange(
                        "(k p) j -> p k j", p=128))
            for c in range(4):
                dma_xq(1, 2 * c, 2)
            nc.sync.dma_start(out=wo_sb,
                              in_=wo[:, :].rearrange("(k p) j -> p k j", p=128))

            nc.gpsimd.memset(zero_sb, 0.0)
            nc.vector.memset(v3_sb, 1.0)  # ones slots; v slots overwritten
            # preload the exp activation table while DMAs stream
            nc.scalar.activation(warm_sb, mask_sb[0:1, 0:1], Exp)

            # ---- warmup: ramp the PE p-state during the DMA lead-in ----
            if N_WARM:
                wp_cm = tc.tile_pool(name="wp", bufs=1, space="PSUM")
                wp = wp_cm.__enter__()
                wps = wp.tile([128, 448], f32, tag="w")
                for _ in range(N_WARM):
                    nc.tensor.matmul(wps, zero_sb[:, 0:128],
                                     zero_sb[:, 0:448],
                                     start=True, stop=True,
                                     skip_group_check=True)
                wp_cm.__exit__(None, None, None)

            # ---- A: K^T = (Wk^T blocks) @ Xk^T, k-major to ride xk DMA ----
            # psS is opened first so psA can close innermost (LIFO pools);
            # emission into psS starts only at phase B.
            psS_cm = tc.tile_pool(name="psS", bufs=1, space="PSUM")
            psS = psS_cm.__enter__()
            psA_cm = tc.tile_pool(name="psA", bufs=1, space="PSUM")
            psA = psA_cm.__enter__()
            for sec0, secw in _chunks(LK, 1024):
                am = [psA.tile([128, 1024], f32, tag="a", bufs=2,
                               name=f"a{m}_{sec0}") for m in range(2)]
                for k in range(KD):
                    for m in range(2):
                        for c0, cw in _chunks(secw, 512):
                            nc.tensor.matmul(
                                am[m][:, c0:c0 + cw],
                                wk_sb[:, k, m * 128:(m + 1) * 128],
                                xk_sb[:, k, sec0 + c0:sec0 + c0 + cw],
                                start=(k == 0), stop=(k == KD - 1))
                for m in range(2):
                    for c0, cw in _chunks(secw, 512):
                        nc.vector.tensor_copy(
                            kt_sb[:, m, sec0 + c0:sec0 + c0 + cw],
                            am[m][:, c0:c0 + cw])

            # ---- B half 0: Q^T cols 0:1024, k-major to ride xq DMA ----
            qp = [psS.tile([128, 1024], f32, tag="s", bufs=2, name=f"q{m}")
                  for m in range(2)]
            for k in range(KD):
                for m in range(2):
                    for cq in range(2):
                        nc.tensor.matmul(
                            qp[m][:, cq * 512:(cq + 1) * 512],
                            wq_sb[:, k, m * 128:(m + 1) * 128],
                            xq_sb[:, k, cq * 512:(cq + 1) * 512],
                            start=(k == 0), stop=(k == KD - 1))
            for m in range(2):
                nc.scalar.copy(qt_sb[:, m, 0:1024], qp[m])

            # ---- V-proj: per key tile, rides the per-tile xv DMA ----
            for tp in range(0, KT, 2):
                nj = min(2, KT - tp)
                vt = psA.tile([128, 2, 4, HD], f32, tag="a", bufs=2,
                              name=f"v{tp}")
                for j in range(nj):
                    t = tp + j
                    for k in range(KD):
                        nc.tensor.matmul(vt[:, j], xv_sb[:, k, t, :],
                                         wv_sb[:, k, :],
                                         start=(k == 0), stop=(k == KD - 1),
                                         skip_group_check=True)
                for j in range(nj):
                    t = tp + j
                    nc.vector.tensor_copy(v3_sb[:, :, t, 0, :], vt[:, j])
            psA_cm.__exit__(None, None, None)

            psX_cm = tc.tile_pool(name="psX", bufs=1, space="PSUM")
            psX = psX_cm.__enter__()
            psC_cm = tc.tile_pool(name="psC", bufs=1, space="PSUM")
            psC = psC_cm.__enter__()
            ptp_cm = tc.tile_pool(name="ptp", bufs=1)
            ptp = ptp_cm.__enter__()
            otp_cm = tc.tile_pool(name="otp", bufs=1)
            otp = otp_cm.__enter__()
            rcp_cm = tc.tile_pool(name="rcp", bufs=1)
            rcp = rcp_cm.__enter__()

            # Q-proj matmuls for half 1 (cols 1024:2048), injected one at a
            # time into attention-half-0 slack; rides the late xq DMA.
            def bh1_quarter(qq):
                xp = psX.tile([128, 2, 512], f32, tag="x", name=f"b1_{qq}")
                thunks = []
                for k in range(KD):
                    for m in range(2):
                        def mm(k=k, m=m):
                            nc.tensor.matmul(
                                xp[:, m, :],
                                wq_sb[:, k, m * 128:(m + 1) * 128],
                                xq_sb[:, k, qq * 512:(qq + 1) * 512],
                                start=(k == 0), stop=(k == KD - 1),
                                skip_group_check=True)
                        thunks.append(mm)

                def evac(qq=qq, xp=xp):
                    nc.vector.tensor_copy(qt_sb[:, :, qq * 512:(qq + 1) * 512],
                                          xp)
                thunks.append(evac)
                return thunks

            # out-proj for one query tile; evac split DVE/Pool (half 0,
            # injected under attention half 1) or ACT/DVE (half 1 tail).
            def d_tile(qi, in_s_pool, tail):
                if in_s_pool:
                    ov = psS.tile([128, 2, 512], f32, tag="s", bufs=2,
                                  name=f"o{qi}")
                else:
                    ov = psX.tile([128, 2, 512], f32, tag="x", name=f"o{qi}")
                thunks = []
                for kk in range(2):
                    for n in range(2):
                        def mm(n=n, kk=kk):
                            nc.tensor.matmul(
                                ov[:, n, :],
                                ctxT_sb[:, kk, qi * 128:(qi + 1) * 128],
                                wo_sb[:, kk, n * 512:(n + 1) * 512],
                                start=(kk == 0), stop=(kk == 1),
                                skip_group_check=True)
                        thunks.append(mm)

                def evac():
                    ot = otp.tile([128, 2, 512], f16, tag="ot", bufs=4,
                                  name=f"ot{qi}")
                    if qi == 15:
                        # pipeline the final tile's evac halves with its DMA
                        nc.scalar.copy(ot[:, 0, :], ov[:, 0, :])
                        nc.sync.dma_start(
                            out=out[qi * 128:(qi + 1) * 128, 0:512],
                            in_=ot[:, 0, :])
                        nc.vector.tensor_copy(ot[:, 1, :], ov[:, 1, :])
                        nc.sync.dma_start(
                            out=out[qi * 128:(qi + 1) * 128, 512:1024],
                            in_=ot[:, 1, :])
                        return
                    if tail:
                        nc.scalar.copy(ot[:, 0, :], ov[:, 0, :])
                        nc.vector.tensor_copy(ot[:, 1, :], ov[:, 1, :])
                    elif qi % 2 == 0:
                        nc.scalar.copy(ot, ov)
                    else:
                        nc.vector.tensor_copy(ot, ov)
                    nc.sync.dma_start(out=out[qi * 128:(qi + 1) * 128, :],
                                      in_=ot.rearrange("p a b -> p (a b)"))
                thunks.append(evac)
                return thunks

            # ---- attention block for (half, head): scores -> exp -> ctx ----
            def c_block(half, hh, inject, q0=None, qw=1024):
                mt, mo = hh // 2, 64 * (hh % 2)
                if q0 is None:
                    q0 = half * 1024
                ncq = qw // 512
                ctx = [psC.tile([128, 512], f32, tag="c", bufs=2,
                                name=f"c{half}_{hh}_{q0}_{cq}")
                       for cq in range(ncq)]
                sts, pts = [], []

                def s_step(t):
                    st = psS.tile([128, qw], f32, tag="s", bufs=2,
                                  name=f"s{half}_{hh}_{q0}_{t}")
                    for cq in range(ncq):
                        nc.tensor.matmul(
                            st[:, cq * 512:(cq + 1) * 512],
                            kt_sb[mo:mo + 64, mt, t * 128:(t + 1) * 128],
                            qt_sb[mo:mo + 64, mt,
                                  q0 + cq * 512:q0 + (cq + 1) * 512],
                            start=True, stop=True)
                    sts.append(st)

                def e_step(t):
                    pt = ptp.tile([128, qw], f16, tag="pt", bufs=8,
                                  name=f"p{half}_{hh}_{q0}_{t}")
                    nc.scalar.activation(pt, sts[t], Exp,
                                         bias=mask_sb[:, t:t + 1], scale=0.125)
                    pts.append(pt)

                def c_step(t):
                    for cq in range(ncq):
                        nc.tensor.matmul(
                            ctx[cq], v3_sb[:, hh, t, :, :],
                            pts[t][:, cq * 512:(cq + 1) * 512],
                            start=(t == 0), stop=(t == KT - 1),
                            skip_group_check=True)

                def drip(n):
                    for _ in range(n):
                        if inject:
                            inject.pop(0)()

                s_step(0)
                for t in range(1, KT):
                    s_step(t)
                    e_step(t - 1)
                    c_step(t - 1)
                    drip(2)
                e_step(KT - 1)
                c_step(KT - 1)
                drip(2)
                for cq in range(ncq):
                    rc = rcp.tile([64, 512], f32, tag="r", bufs=4,
                                  name=f"r{half}_{hh}_{q0}_{cq}")
                    nc.vector.reciprocal(rc, ctx[cq][64:128, :])
                    nc.vector.tensor_mul(
                        ctxT_sb[mo:mo + 64, mt,
                                q0 + cq * 512:q0 + (cq + 1) * 512],
                        ctx[cq][0:64, :], rc)

            # ---- attention half 0, with Q-proj half 1 injected ----
            inject0 = bh1_quarter(2) + bh1_quarter(3)
            for hh in range(4):
                c_block(0, hh, inject0)
            for th in inject0:  # anything not yet dripped
                th()

            # ---- attention half 1, with out-proj half 0 injected; the
            # last head runs as two 512-query blocks so the first half-1
            # out-proj tiles unlock while its second half still computes ----
            inject1 = []
            for qi in range(8):
                inject1 += d_tile(qi, in_s_pool=False, tail=False)
            for hh in range(3):
                c_block(1, hh, inject1)
            c_block(1, 3, inject1, q0=1024, qw=512)
            c_block(1, 3, inject1, q0=1536, qw=512)
            for th in inject1:
                th()

            # ---- out-proj tail (s,s,x PSUM pattern: 3-deep pipeline) ----
            for qi in range(8, 16):
                for th in d_tile(qi, in_s_pool=((qi - 8) % 3 != 2),
                                 tail=True):
                    th()

            rcp_cm.__exit__(None, None, None)
            otp_cm.__exit__(None, None, None)
            ptp_cm.__exit__(None, None, None)
            psC_cm.__exit__(None, None, None)
            psX_cm.__exit__(None, None, None)
            psS_cm.__exit__(None, None, None)
    nc.compile()
    return nc


def kernel(**inputs):
    global last_results, last_exec_wall_s
    from concourse.bass_utils import run_bass_kernel_spmd

    # BASS_TRACE needs the axon NTFF hook; disable tracing when the hook
    # module is unavailable so a stray env var cannot crash the run.
    if os.environ.get("BASS_TRACE"):
        try:
            from antenv import axon_hooks  # noqa: F401
        except Exception:
            os.environ["BASS_NEVER_TRACE"] = "1"

    q = np.asarray(inputs["queries"], dtype=np.float32)
    kx = np.asarray(inputs["keys"], dtype=np.float32)
    vx = np.asarray(inputs["values"], dtype=np.float32)
    vl = np.asarray(inputs["valid_lens"], dtype=np.int64).reshape(B)
    Wq = np.asarray(inputs["Wq"], dtype=np.float32)
    Wk = np.asarray(inputs["Wk"], dtype=np.float32)
    Wv = np.asarray(inputs["Wv"], dtype=np.float32)
    Wo = np.asarray(inputs["Wo"], dtype=np.float32)
    assert q.shape == (B, SQ, D) and kx.shape == (B, SK, D) and vx.shape == (B, SK, D)

    lens = np.clip(vl, 1, SK)
    lmax = int(lens.max())
    KT = (lmax + 127) // 128
    LK = KT * 128

    if KT not in _NC_CACHE:
        _NC_CACHE[KT] = _build(KT)
    nc = _NC_CACHE[KT]

    in_maps = []
    for c in range(N_CORES):
        b, hg = c // 4, c % 4
        cols = slice(DL * hg, DL * (hg + 1))
        m = np.where(np.arange(LK) < lens[b], 0.0, NEG).astype(np.float32)
        in_maps.append({
            "xqT": np.ascontiguousarray(q[b].T.astype(np.float16)),
            "xkT": np.ascontiguousarray(kx[b, :LK].T.astype(np.float16)),
            "xvT": np.ascontiguousarray(vx[b, :LK].T.astype(np.float16)),
            "wq": np.ascontiguousarray(Wq[:, cols].astype(np.float16)),
            "wk": np.ascontiguousarray(Wk[:, cols].astype(np.float16)),
            "wv": np.ascontiguousarray(Wv[:, cols].astype(np.float16)),
            "wo": np.ascontiguousarray(Wo[cols, :].astype(np.float16)),
            "mask": np.ascontiguousarray(m.reshape(KT, 128).T),
        })

    t0 = time.perf_counter()
    res = run_bass_kernel_spmd(nc, in_maps, core_ids=list(range(N_CORES)))
    last_exec_wall_s = time.perf_counter() - t0
    last_results = res

    outs = [res.results[c]["out"].astype(np.float32) for c in range(N_CORES)]
    full = np.stack([outs[0] + outs[1] + outs[2] + outs[3],
                     outs[4] + outs[5] + outs[6] + outs[7]])
    return full.astype(np.float32)
